# revision 21
# baseline (speedup 1.0000x reference)
"""Trainium2 (8 NeuronCores) kernel for batched multi-head causal attention.

Problem: q,k,v [4, 16, 2048, 64] f32, attention_mask [4, 1, 2048] (all ones).
Reference: softmax((q@k^T + causal_mask) * 1/sqrt(64)) @ v, rows masked above
the diagonal.

Sharding: pure data/head parallelism. B*H = 64 heads, 8 heads per core; core c
takes flattened heads [8c, 8c+8).  No cross-core communication.

Per-core algorithm (per head, S=2048, D=64):
  - Q^T and K^T ([64 d, 2048 s], d on partitions) are produced by PE
    transposes of natural [128, 64] tiles; two heads are packed per [128, 128]
    transpose (head A -> out partitions 0:64, head B -> 64:128) so the
    PSUM->SBUF copies run at full 128-lane DVE width.
  - Scores are computed transposed, S^T[s, l] (s on partitions), so the PV
    matmul consumes P^T directly as the moving operand and V natural as
    stationary.  All matmuls use float32r (full-rate fp32).
  - exp on ScalarE with the 1/sqrt(D) scale folded into the activation's
    free affine.  Causal masking applied post-exp via gpsimd affine_select
    (fill 0.0 above the diagonal) on the diagonal s-tiles.
  - Softmax denominator comes free from an appended ones-column on V
    (PV stationary is [128, 65]); output is computed unnormalized, then
    transposed back (PE) and scaled by the reciprocal row-sum (DVE).
"""

import numpy as np
from contextlib import ExitStack

# problem shape (hardcoded; kernel.py must be self-contained)
B, H, S, D = 4, 16, 2048, 64
NCORES = 8
NH = (B * H) // NCORES   # 8 heads per core
ST = 128                 # s-tile (key) rows per matmul
NST = S // ST            # 16 s-tiles
LT = 512                 # l-tile (query) columns per psum bank
NLT = S // LT            # 4 l-tiles
GRP = 3                  # s-tiles per exp group (3 psum banks)
SCALE = 1.0 / float(np.sqrt(D))

_CACHE = {}


def _build_nc(reps=1):
    import concourse.bacc as bacc
    import concourse.bass as bass
    import concourse.mybir as mybir
    import concourse.tile as tile
    from concourse.masks import make_identity

    F32 = mybir.dt.float32
    F32R = mybir.dt.float32r
    BF16 = mybir.dt.bfloat16
    EXP = mybir.ActivationFunctionType.Exp

    nc = bacc.Bacc("TRN2", target_bir_lowering=False, debug=False, num_devices=NCORES)

    q_d = nc.dram_tensor("q", [NH, S, D], F32, kind="ExternalInput")
    k_d = nc.dram_tensor("k", [NH, S, D], F32, kind="ExternalInput")
    v_d = nc.dram_tensor("v", [NH, S, D], F32, kind="ExternalInput")
    o_d = nc.dram_tensor("out", [NH, S, D], F32, kind="ExternalOutput")

    with tile.TileContext(nc) as tc, ExitStack() as ctx:
        const = ctx.enter_context(tc.tile_pool(name="const", bufs=1))
        nat = ctx.enter_context(tc.tile_pool(name="nat", bufs=2))
        natv = ctx.enter_context(tc.tile_pool(name="natv", bufs=4))
        qkt = ctx.enter_context(tc.tile_pool(name="qkt", bufs=2))
        pts = ctx.enter_context(tc.tile_pool(name="pts", bufs=3))
        ovs = ctx.enter_context(tc.tile_pool(name="ovs", bufs=2))
        rts = ctx.enter_context(tc.tile_pool(name="rts", bufs=2))
        osb = ctx.enter_context(tc.tile_pool(name="osb", bufs=2))
        psc = ctx.enter_context(tc.tile_pool(name="psc", bufs=2, space="PSUM"))
        ppv = ctx.enter_context(tc.tile_pool(name="ppv", bufs=2, space="PSUM"))

        ident = const.tile([128, 128], F32)
        make_identity(nc, ident[:])
        identb = const.tile([128, 128], mybir.dt.bfloat16, tag="identb")
        make_identity(nc, identb[:])

        import contextlib

        _eng = mybir.EngineType
        loop = (
            tc.For_i(0, reps, 1,
                     hint_engines=(_eng.PE, _eng.DVE, _eng.Activation, _eng.Pool, _eng.SP))
            if reps > 1
            else contextlib.nullcontext()
        )
        with loop:
            _emit_body(nc, tc, mybir, F32, BF16, EXP,
                       const, nat, natv, qkt, pts, ovs, rts, osb,
                       psc, ppv, ident, identb, q_d, k_d, v_d, o_d)

    nc.compile()
    return nc


def _emit_body(nc, tc, mybir, F32, BF16, EXP,
               const, nat, natv, qkt, pts, ovs, rts, osb,
               psc, ppv, ident, identb, q_d, k_d, v_d, o_d):
    if True:
        for pair in range(NH // 2):
            hA, hB = 2 * pair, 2 * pair + 1

            # ---- load q/k with heads A|B packed along d ----------------------
            # [128, t, 0:64] = head A, [128, t, 64:128] = head B.  The [128,128]
            # PE transpose of one t-slice then lands head A on out partitions
            # 0:64 and head B on 64:128 with PSUM base partition 0 (the only
            # legal transpose output position).
            def load_pair(src, tag):
                raw = nat.tile([128, NST, 2 * D], F32, tag=tag + "f")
                for i, h in enumerate((hA, hB)):
                    nc.sync.dma_start(
                        out=raw[:, :, i * D : (i + 1) * D],
                        in_=src.ap()[h].rearrange("(t p) d -> p t d", p=128),
                    )
                t = nat.tile([128, NST, 2 * D], BF16, tag=tag)
                nc.vector.tensor_copy(t[:], raw[:])
                return t

            qn = load_pair(q_d, "qn")
            kn = load_pair(k_d, "kn")

            def load_v(h):
                raw = natv.tile([128, NST, D], F32, tag="vn")
                nc.sync.dma_start(
                    out=raw[:],
                    in_=v_d.ap()[h].rearrange("(t p) d -> p t d", p=128),
                )
                t = natv.tile([128, NST, D + 1], BF16, tag="vr")
                nc.vector.tensor_copy(t[:, :, 0:D], raw[:])
                nc.gpsimd.memset(t[:, :, D : D + 1], 1.0)
                return t

            vA = load_v(hA)
            vB = load_v(hB)

            # ---- transpose q/k into [64, 2048] per head, packed A|B ----------
            QT = qkt.tile([128, S], BF16, tag="QT")
            KT = qkt.tile([128, S], BF16, tag="KT")
            for dst, srct in ((QT, qn), (KT, kn)):
                done = 0
                while done < NST:
                    n = min(6, NST - done)
                    stg = psc.tile([128, GRP * LT], BF16, tag="sc")
                    for j in range(n):
                        nc.tensor.transpose(
                            stg[:, 128 * j : 128 * (j + 1)], srct[:, done + j, :], identb[:]
                        )
                    nc.vector.tensor_copy(
                        dst[:, ST * done : ST * (done + n)], stg[:, 0 : 128 * n]
                    )
                    done += n

            # ---- attention per head ------------------------------------------
            for h, rb, vt in ((hA, 0, vA), (hB, 64, vB)):
                outsb = osb.tile([128, NST, D], F32, tag="outsb")
                for lt in range(NLT):
                    l0 = lt * LT
                    n_s = 4 * lt + 4  # visible s-tiles for this l-tile
                    pvt = ppv.tile([D + 1, LT], F32, tag="pv")
                    done = 0
                    while done < n_s:
                        g = min(GRP, n_s - done)
                        sc = psc.tile([128, GRP * LT], F32, tag="sc")
                        pt = pts.tile([128, GRP * LT], BF16, tag="pt")

                        def offof(t):
                            c0 = t * ST - l0
                            return c0 if c0 in (128, 256, 384) else 0

                        for j in range(g):
                            t = done + j
                            off = offof(t)
                            nc.tensor.matmul(
                                sc[:, LT * j + off : LT * (j + 1)],
                                lhsT=KT[rb : rb + 64, t * ST : (t + 1) * ST],
                                rhs=QT[rb : rb + 64, l0 + off : l0 + LT],
                                start=True,
                                stop=True,
                            )
                        # exp in segments that skip the never-written
                        # (fully-masked) column ranges of diagonal s-tiles
                        segs, cur = [], 0
                        for j in range(g):
                            off = offof(done + j)
                            if off:
                                if cur < LT * j:
                                    segs.append((cur, LT * j))
                                cur = LT * j + off
                        segs.append((cur, LT * g))
                        for a, b in segs:
                            nc.scalar.activation(
                                pt[:, a:b], sc[:, a:b], EXP, scale=SCALE
                            )
                        for j in range(g):
                            t = done + j
                            c0 = t * ST - l0
                            off = offof(t)
                            if c0 >= 0:  # diagonal s-tile: zero where s > l
                                # keep where col - i >= 0 relative to the
                                # diagonal start (is_le broken; is_ge works)
                                nc.gpsimd.affine_select(
                                    out=pt[:, LT * j + off : LT * (j + 1)],
                                    in_=pt[:, LT * j + off : LT * (j + 1)],
                                    compare_op=mybir.AluOpType.is_ge,
                                    fill=0.0,
                                    base=-(c0 - off),
                                    channel_multiplier=-1,
                                    pattern=[[1, LT - off]],
                                )
                        for j in range(g):
                            t = done + j
                            off = offof(t)
                            nc.tensor.matmul(
                                pvt[:, off:LT],
                                lhsT=vt[:, t, :],
                                rhs=pt[:, LT * j + off : LT * (j + 1)],
                                start=(t == 0),
                                stop=(t == n_s - 1),
                            )
                        done += g

                    # epilogue: transpose back, normalize by row-sum
                    ovt = ovs.tile([D + 1, LT], BF16, tag="ov")
                    nc.vector.tensor_copy(ovt[:], pvt[:])
                    ost = ppv.tile([128, 4 * (D + 4)], BF16, tag="pv")
                    for j in range(4):
                        nc.tensor.transpose(
                            ost[:, (D + 4) * j : (D + 4) * j + D + 1],
                            ovt[:, 128 * j : 128 * (j + 1)],
                            identb[0 : D + 1, 0 : D + 1],
                        )
                    osr = ost[:].rearrange("p (j c) -> p j c", c=D + 4)
                    rt = rts.tile([128, 4], F32, tag="rt")
                    nc.vector.reciprocal(rt[:], osr[:, :, D])
                    nc.vector.tensor_mul(
                        outsb[:, 4 * lt : 4 * lt + 4, :],
                        osr[:, :, 0:D],
                        rt[:].unsqueeze(2).to_broadcast((128, 4, D)),
                    )
                nc.sync.dma_start(
                    out=o_d.ap()[h].rearrange("(c p) d -> p c d", p=128),
                    in_=outsb[:],
                )


def get_nc(reps=1):
    if reps not in _CACHE:
        _CACHE[reps] = _build_nc(reps)
    return _CACHE[reps]


def make_in_maps(q, k, v):
    q = np.ascontiguousarray(np.asarray(q, dtype=np.float32).reshape(B * H, S, D))
    k = np.ascontiguousarray(np.asarray(k, dtype=np.float32).reshape(B * H, S, D))
    v = np.ascontiguousarray(np.asarray(v, dtype=np.float32).reshape(B * H, S, D))
    maps = []
    for c in range(NCORES):
        sl = slice(c * NH, (c + 1) * NH)
        maps.append(
            {
                "q": np.ascontiguousarray(q[sl]),
                "k": np.ascontiguousarray(k[sl]),
                "v": np.ascontiguousarray(v[sl]),
            }
        )
    return maps


def kernel(q, k, v, attention_mask=None, **_ignored):
    """Full inputs in, full output out. attention_mask is all-ones by
    construction in this problem and drops out of the math."""
    from concourse.bass_utils import run_bass_kernel_spmd

    nc = get_nc()
    res = run_bass_kernel_spmd(nc, make_in_maps(q, k, v), core_ids=list(range(NCORES)))
    out = np.concatenate([res.results[c]["out"] for c in range(NCORES)], axis=0)
    return out.reshape(B, H, S, D).astype(np.float32)


# revision 22
# speedup vs baseline: 1.0788x; 1.0788x over previous
"""Trainium2 (8 NeuronCores) kernel for batched multi-head causal attention.

Problem: q,k,v [4, 16, 2048, 64] f32, attention_mask [4, 1, 2048] (all ones).
Reference: softmax((q@k^T + causal_mask) * 1/sqrt(64)) @ v, rows masked above
the diagonal.

Sharding: pure data/head parallelism. B*H = 64 heads, 8 heads per core; core c
takes flattened heads [8c, 8c+8).  No cross-core communication.

Per-core algorithm (per head, S=2048, D=64):
  - Q^T and K^T ([64 d, 2048 s], d on partitions) are produced by PE
    transposes of natural [128, 64] tiles; two heads are packed per [128, 128]
    transpose (head A -> out partitions 0:64, head B -> 64:128) so the
    PSUM->SBUF copies run at full 128-lane DVE width.
  - Scores are computed transposed, S^T[s, l] (s on partitions), so the PV
    matmul consumes P^T directly as the moving operand and V natural as
    stationary.  All matmuls use float32r (full-rate fp32).
  - exp on ScalarE with the 1/sqrt(D) scale folded into the activation's
    free affine.  Causal masking applied post-exp via gpsimd affine_select
    (fill 0.0 above the diagonal) on the diagonal s-tiles.
  - Softmax denominator comes free from an appended ones-column on V
    (PV stationary is [128, 65]); output is computed unnormalized, then
    transposed back (PE) and scaled by the reciprocal row-sum (DVE).
"""

import numpy as np
from contextlib import ExitStack

# problem shape (hardcoded; kernel.py must be self-contained)
B, H, S, D = 4, 16, 2048, 64
NCORES = 8
NH = (B * H) // NCORES   # 8 heads per core
ST = 128                 # s-tile (key) rows per matmul
NST = S // ST            # 16 s-tiles
LT = 512                 # l-tile (query) columns per psum bank
NLT = S // LT            # 4 l-tiles
GRP = 3                  # s-tiles per exp group (3 psum banks)
SCALE = 1.0 / float(np.sqrt(D))

_CACHE = {}


def _build_nc(reps=1):
    import concourse.bacc as bacc
    import concourse.bass as bass
    import concourse.mybir as mybir
    import concourse.tile as tile
    from concourse.masks import make_identity

    F32 = mybir.dt.float32
    F32R = mybir.dt.float32r
    BF16 = mybir.dt.bfloat16
    EXP = mybir.ActivationFunctionType.Exp

    nc = bacc.Bacc("TRN2", target_bir_lowering=False, debug=False, num_devices=NCORES)

    q_d = nc.dram_tensor("q", [NH, S, D], F32, kind="ExternalInput")
    k_d = nc.dram_tensor("k", [NH, S, D], F32, kind="ExternalInput")
    v_d = nc.dram_tensor("v", [NH, S, D], F32, kind="ExternalInput")
    o_d = nc.dram_tensor("out", [NH, S, D], F32, kind="ExternalOutput")

    with tile.TileContext(nc) as tc, ExitStack() as ctx:
        const = ctx.enter_context(tc.tile_pool(name="const", bufs=1))
        nat = ctx.enter_context(tc.tile_pool(name="nat", bufs=2))
        natv = ctx.enter_context(tc.tile_pool(name="natv", bufs=4))
        qkt = ctx.enter_context(tc.tile_pool(name="qkt", bufs=2))
        pts = ctx.enter_context(tc.tile_pool(name="pts", bufs=3))
        ovs = ctx.enter_context(tc.tile_pool(name="ovs", bufs=2))
        rts = ctx.enter_context(tc.tile_pool(name="rts", bufs=2))
        osb = ctx.enter_context(tc.tile_pool(name="osb", bufs=2))
        psc = ctx.enter_context(tc.tile_pool(name="psc", bufs=2, space="PSUM"))
        ppv = ctx.enter_context(tc.tile_pool(name="ppv", bufs=2, space="PSUM"))

        ident = const.tile([128, 128], F32)
        make_identity(nc, ident[:])
        identb = const.tile([128, 128], mybir.dt.bfloat16, tag="identb")
        make_identity(nc, identb[:])

        import contextlib

        _eng = mybir.EngineType
        loop = (
            tc.For_i(0, reps, 1,
                     hint_engines=(_eng.PE, _eng.DVE, _eng.Activation, _eng.Pool, _eng.SP))
            if reps > 1
            else contextlib.nullcontext()
        )
        with loop:
            _emit_body(nc, tc, mybir, F32, BF16, EXP,
                       const, nat, natv, qkt, pts, ovs, rts, osb,
                       psc, ppv, ident, identb, q_d, k_d, v_d, o_d)

    nc.compile()
    return nc


def _emit_body(nc, tc, mybir, F32, BF16, EXP,
               const, nat, natv, qkt, pts, ovs, rts, osb,
               psc, ppv, ident, identb, q_d, k_d, v_d, o_d):
    if True:
        for pair in range(NH // 2):
            hA, hB = 2 * pair, 2 * pair + 1

            # ---- load q/k with heads A|B packed along d ----------------------
            # [128, t, 0:64] = head A, [128, t, 64:128] = head B.  The [128,128]
            # PE transpose of one t-slice then lands head A on out partitions
            # 0:64 and head B on 64:128 with PSUM base partition 0 (the only
            # legal transpose output position).
            def load_pair(src, tag):
                raw = nat.tile([128, NST, 2 * D], F32, tag=tag + "f")
                for i, h in enumerate((hA, hB)):
                    nc.sync.dma_start(
                        out=raw[:, :, i * D : (i + 1) * D],
                        in_=src.ap()[h].rearrange("(t p) d -> p t d", p=128),
                    )
                t = nat.tile([128, NST, 2 * D], BF16, tag=tag)
                nc.scalar.copy(t[:], raw[:])
                return t

            qn = load_pair(q_d, "qn")
            kn = load_pair(k_d, "kn")

            def load_v(h):
                raw = natv.tile([128, NST, D], F32, tag="vn")
                nc.sync.dma_start(
                    out=raw[:],
                    in_=v_d.ap()[h].rearrange("(t p) d -> p t d", p=128),
                )
                t = natv.tile([128, NST, D + 1], BF16, tag="vr")
                nc.scalar.copy(t[:, :, 0:D], raw[:])
                nc.gpsimd.memset(t[:, :, D : D + 1], 1.0)
                return t

            vA = load_v(hA)
            vB = load_v(hB)

            # ---- transpose q/k into [64, 2048] per head, packed A|B ----------
            QT = qkt.tile([128, S], BF16, tag="QT")
            KT = qkt.tile([128, S], BF16, tag="KT")
            for dst, srct in ((QT, qn), (KT, kn)):
                done = 0
                while done < NST:
                    n = min(6, NST - done)
                    stg = psc.tile([128, GRP * LT], BF16, tag="sc")
                    for j in range(n):
                        nc.tensor.transpose(
                            stg[:, 128 * j : 128 * (j + 1)], srct[:, done + j, :], identb[:]
                        )
                    nc.vector.tensor_copy(
                        dst[:, ST * done : ST * (done + n)], stg[:, 0 : 128 * n]
                    )
                    done += n

            # ---- attention per head ------------------------------------------
            for h, rb, vt in ((hA, 0, vA), (hB, 64, vB)):
                outsb = osb.tile([128, NST, D], F32, tag="outsb")
                for lt in range(NLT):
                    l0 = lt * LT
                    n_s = 4 * lt + 4  # visible s-tiles for this l-tile
                    pvt = ppv.tile([D + 1, LT], F32, tag="pv")
                    done = 0
                    while done < n_s:
                        g = min(GRP, n_s - done)
                        sc = psc.tile([128, GRP * LT], F32, tag="sc")
                        pt = pts.tile([128, GRP * LT], BF16, tag="pt")

                        def offof(t):
                            c0 = t * ST - l0
                            return c0 if c0 in (128, 256, 384) else 0

                        for j in range(g):
                            t = done + j
                            off = offof(t)
                            nc.tensor.matmul(
                                sc[:, LT * j + off : LT * (j + 1)],
                                lhsT=KT[rb : rb + 64, t * ST : (t + 1) * ST],
                                rhs=QT[rb : rb + 64, l0 + off : l0 + LT],
                                start=True,
                                stop=True,
                            )
                        # exp in segments that skip the never-written
                        # (fully-masked) column ranges of diagonal s-tiles
                        segs, cur = [], 0
                        for j in range(g):
                            off = offof(done + j)
                            if off:
                                if cur < LT * j:
                                    segs.append((cur, LT * j))
                                cur = LT * j + off
                        segs.append((cur, LT * g))
                        for a, b in segs:
                            nc.scalar.activation(
                                pt[:, a:b], sc[:, a:b], EXP, scale=SCALE
                            )
                        for j in range(g):
                            t = done + j
                            c0 = t * ST - l0
                            off = offof(t)
                            if c0 >= 0:  # diagonal s-tile: zero where s > l
                                # keep where col - i >= 0 relative to the
                                # diagonal start (is_le broken; is_ge works)
                                nc.gpsimd.affine_select(
                                    out=pt[:, LT * j + off : LT * (j + 1)],
                                    in_=pt[:, LT * j + off : LT * (j + 1)],
                                    compare_op=mybir.AluOpType.is_ge,
                                    fill=0.0,
                                    base=-(c0 - off),
                                    channel_multiplier=-1,
                                    pattern=[[1, LT - off]],
                                )
                        for j in range(g):
                            t = done + j
                            off = offof(t)
                            nc.tensor.matmul(
                                pvt[:, off:LT],
                                lhsT=vt[:, t, :],
                                rhs=pt[:, LT * j + off : LT * (j + 1)],
                                start=(t == 0),
                                stop=(t == n_s - 1),
                            )
                        done += g

                    # epilogue: transpose back, normalize by row-sum
                    ovt = ovs.tile([D + 1, LT], BF16, tag="ov")
                    nc.vector.tensor_copy(ovt[:], pvt[:])
                    ost = ppv.tile([128, 4 * (D + 4)], BF16, tag="pv")
                    for j in range(4):
                        nc.tensor.transpose(
                            ost[:, (D + 4) * j : (D + 4) * j + D + 1],
                            ovt[:, 128 * j : 128 * (j + 1)],
                            identb[0 : D + 1, 0 : D + 1],
                        )
                    osr = ost[:].rearrange("p (j c) -> p j c", c=D + 4)
                    rt = rts.tile([128, 4], F32, tag="rt")
                    nc.vector.reciprocal(rt[:], osr[:, :, D])
                    nc.vector.tensor_mul(
                        outsb[:, 4 * lt : 4 * lt + 4, :],
                        osr[:, :, 0:D],
                        rt[:].unsqueeze(2).to_broadcast((128, 4, D)),
                    )
                nc.sync.dma_start(
                    out=o_d.ap()[h].rearrange("(c p) d -> p c d", p=128),
                    in_=outsb[:],
                )


def get_nc(reps=1):
    if reps not in _CACHE:
        _CACHE[reps] = _build_nc(reps)
    return _CACHE[reps]


def make_in_maps(q, k, v):
    q = np.ascontiguousarray(np.asarray(q, dtype=np.float32).reshape(B * H, S, D))
    k = np.ascontiguousarray(np.asarray(k, dtype=np.float32).reshape(B * H, S, D))
    v = np.ascontiguousarray(np.asarray(v, dtype=np.float32).reshape(B * H, S, D))
    maps = []
    for c in range(NCORES):
        sl = slice(c * NH, (c + 1) * NH)
        maps.append(
            {
                "q": np.ascontiguousarray(q[sl]),
                "k": np.ascontiguousarray(k[sl]),
                "v": np.ascontiguousarray(v[sl]),
            }
        )
    return maps


def kernel(q, k, v, attention_mask=None, **_ignored):
    """Full inputs in, full output out. attention_mask is all-ones by
    construction in this problem and drops out of the math."""
    from concourse.bass_utils import run_bass_kernel_spmd

    nc = get_nc()
    res = run_bass_kernel_spmd(nc, make_in_maps(q, k, v), core_ids=list(range(NCORES)))
    out = np.concatenate([res.results[c]["out"] for c in range(NCORES)], axis=0)
    return out.reshape(B, H, S, D).astype(np.float32)


# revision 24
# speedup vs baseline: 2.6678x; 2.4729x over previous
"""Trainium2 (8 NeuronCores) kernel for batched multi-head causal attention.

Problem: q,k,v [4, 16, 2048, 64] f32, attention_mask [4, 1, 2048] (all ones).
Reference: softmax((q@k^T + causal_mask) * 1/sqrt(64)) @ v, rows masked above
the diagonal.

Sharding: pure data/head parallelism. B*H = 64 heads, 8 heads per core; core c
takes flattened heads [8c, 8c+8).  No cross-core communication.

Per-core algorithm (per head, S=2048, D=64):
  - Q^T and K^T ([64 d, 2048 s], d on partitions) are produced by PE
    transposes of natural [128, 64] tiles; two heads are packed per [128, 128]
    transpose (head A -> out partitions 0:64, head B -> 64:128) so the
    PSUM->SBUF copies run at full 128-lane DVE width.
  - Scores are computed transposed, S^T[s, l] (s on partitions), so the PV
    matmul consumes P^T directly as the moving operand and V natural as
    stationary.  All matmuls use float32r (full-rate fp32).
  - exp on ScalarE with the 1/sqrt(D) scale folded into the activation's
    free affine.  Causal masking applied post-exp via gpsimd affine_select
    (fill 0.0 above the diagonal) on the diagonal s-tiles.
  - Softmax denominator comes free from an appended ones-column on V
    (PV stationary is [128, 65]); output is computed unnormalized, then
    transposed back (PE) and scaled by the reciprocal row-sum (DVE).
"""

import numpy as np
from contextlib import ExitStack

# problem shape (hardcoded; kernel.py must be self-contained)
B, H, S, D = 4, 16, 2048, 64
NCORES = 8
NH = (B * H) // NCORES   # 8 heads per core
ST = 128                 # s-tile (key) rows per matmul
NST = S // ST            # 16 s-tiles
LT = 512                 # l-tile (query) columns per psum bank
NLT = S // LT            # 4 l-tiles
GRP = 2                  # s-tiles per exp group (2 psum banks)
SCALE = 1.0 / float(np.sqrt(D))

_CACHE = {}


def _build_nc(reps=1):
    import concourse.bacc as bacc
    import concourse.bass as bass
    import concourse.mybir as mybir
    import concourse.tile as tile
    from concourse.masks import make_identity

    F32 = mybir.dt.float32
    F32R = mybir.dt.float32r
    BF16 = mybir.dt.bfloat16
    EXP = mybir.ActivationFunctionType.Exp

    nc = bacc.Bacc("TRN2", target_bir_lowering=False, debug=False, num_devices=NCORES)

    q_d = nc.dram_tensor("q", [NH, S, D], F32, kind="ExternalInput")
    k_d = nc.dram_tensor("k", [NH, S, D], F32, kind="ExternalInput")
    v_d = nc.dram_tensor("v", [NH, S, D], F32, kind="ExternalInput")
    o_d = nc.dram_tensor("out", [NH, S, D], F32, kind="ExternalOutput")

    with tile.TileContext(nc) as tc, ExitStack() as ctx:
        const = ctx.enter_context(tc.tile_pool(name="const", bufs=1))
        nat = ctx.enter_context(tc.tile_pool(name="nat", bufs=2))
        natv = ctx.enter_context(tc.tile_pool(name="natv", bufs=4))
        qkt = ctx.enter_context(tc.tile_pool(name="qkt", bufs=2))
        pts = ctx.enter_context(tc.tile_pool(name="pts", bufs=3))
        ovs = ctx.enter_context(tc.tile_pool(name="ovs", bufs=2))
        rts = ctx.enter_context(tc.tile_pool(name="rts", bufs=2))
        osb = ctx.enter_context(tc.tile_pool(name="osb", bufs=2))
        psc = ctx.enter_context(tc.tile_pool(name="psc", bufs=3, space="PSUM"))
        drp = ctx.enter_context(tc.tile_pool(name="drp", bufs=2, space="DRAM"))
        ppv = ctx.enter_context(tc.tile_pool(name="ppv", bufs=2, space="PSUM"))

        ident = const.tile([128, 128], F32)
        make_identity(nc, ident[:])
        identb = const.tile([128, 128], mybir.dt.bfloat16, tag="identb")
        make_identity(nc, identb[:])

        import contextlib

        _eng = mybir.EngineType
        loop = (
            tc.For_i(0, reps, 1,
                     hint_engines=(_eng.PE, _eng.DVE, _eng.Activation, _eng.Pool, _eng.SP))
            if reps > 1
            else contextlib.nullcontext()
        )
        with loop:
            _emit_body(nc, tc, mybir, F32, BF16, EXP,
                       const, nat, natv, qkt, pts, ovs, rts, osb,
                       psc, ppv, drp, ident, identb, q_d, k_d, v_d, o_d)

    nc.compile()
    return nc


def _emit_body(nc, tc, mybir, F32, BF16, EXP,
               const, nat, natv, qkt, pts, ovs, rts, osb,
               psc, ppv, drp, ident, identb, q_d, k_d, v_d, o_d):
    if True:
        for pair in range(NH // 2):
            hA, hB = 2 * pair, 2 * pair + 1

            # ---- load q/k with heads A|B packed along d ----------------------
            # [128, t, 0:64] = head A, [128, t, 64:128] = head B.  The [128,128]
            # PE transpose of one t-slice then lands head A on out partitions
            # 0:64 and head B on 64:128 with PSUM base partition 0 (the only
            # legal transpose output position).
            def load_pair(src, tag, dsttag):
                raw = nat.tile([128, NST, 2 * D], F32, tag=tag + "f")
                for i, h in enumerate((hA, hB)):
                    nc.sync.dma_start(
                        out=raw[:, :, i * D : (i + 1) * D],
                        in_=src.ap()[h].rearrange("(t p) d -> p t d", p=128),
                    )
                t = nat.tile([128, NST, 2 * D], BF16, tag=tag)
                nc.scalar.copy(t[:], raw[:])
                # bounce through DRAM and transpose on the way back via the
                # DMA xbar: scratch [2048 s, 128 d(A|B)] -> [128 d(A|B), 2048 s]
                scr = drp.tile([S, 2 * D], BF16, tag=tag + "s")
                nc.sync.dma_start(
                    out=scr[:].rearrange("(t p) d -> p t d", p=128), in_=t[:]
                )
                dst = qkt.tile([128, S], BF16, tag=dsttag)
                nc.sync.dma_start_transpose(out=dst[:], in_=scr[:])
                return dst

            QT = load_pair(q_d, "qn", "QT")
            KT = load_pair(k_d, "kn", "KT")

            def load_v(h):
                raw = natv.tile([128, NST, D], F32, tag="vn")
                nc.sync.dma_start(
                    out=raw[:],
                    in_=v_d.ap()[h].rearrange("(t p) d -> p t d", p=128),
                )
                t = natv.tile([128, NST, D + 1], BF16, tag="vr")
                nc.scalar.copy(t[:, :, 0:D], raw[:])
                nc.gpsimd.memset(t[:, :, D : D + 1], 1.0)
                return t

            vA = load_v(hA)
            vB = load_v(hB)

            # ---- attention per head ------------------------------------------
            for h, rb, vt in ((hA, 0, vA), (hB, 64, vB)):
                outsb = osb.tile([128, NST, D], F32, tag="outsb")
                for lt in range(NLT):
                    l0 = lt * LT
                    n_s = 4 * lt + 4  # visible s-tiles for this l-tile
                    pvt = ppv.tile([D + 1, LT], F32, tag="pv")
                    done = 0
                    while done < n_s:
                        g = min(GRP, n_s - done)
                        sc = psc.tile([128, GRP * LT], F32, tag="sc")
                        pt = pts.tile([128, GRP * LT], BF16, tag="pt")

                        def offof(t):
                            c0 = t * ST - l0
                            return c0 if c0 in (128, 256, 384) else 0

                        for j in range(g):
                            t = done + j
                            off = offof(t)
                            nc.tensor.matmul(
                                sc[:, LT * j + off : LT * (j + 1)],
                                lhsT=KT[rb : rb + 64, t * ST : (t + 1) * ST],
                                rhs=QT[rb : rb + 64, l0 + off : l0 + LT],
                                start=True,
                                stop=True,
                            )
                        # exp in segments that skip the never-written
                        # (fully-masked) column ranges of diagonal s-tiles
                        segs, cur = [], 0
                        for j in range(g):
                            off = offof(done + j)
                            if off:
                                if cur < LT * j:
                                    segs.append((cur, LT * j))
                                cur = LT * j + off
                        segs.append((cur, LT * g))
                        for a, b in segs:
                            nc.scalar.activation(
                                pt[:, a:b], sc[:, a:b], EXP, scale=SCALE
                            )
                        for j in range(g):
                            t = done + j
                            c0 = t * ST - l0
                            off = offof(t)
                            if c0 >= 0:  # diagonal s-tile: zero where s > l
                                # keep where col - i >= 0 relative to the
                                # diagonal start (is_le broken; is_ge works)
                                nc.gpsimd.affine_select(
                                    out=pt[:, LT * j + off : LT * (j + 1)],
                                    in_=pt[:, LT * j + off : LT * (j + 1)],
                                    compare_op=mybir.AluOpType.is_ge,
                                    fill=0.0,
                                    base=-(c0 - off),
                                    channel_multiplier=-1,
                                    pattern=[[1, LT - off]],
                                )
                        for j in range(g):
                            t = done + j
                            off = offof(t)
                            nc.tensor.matmul(
                                pvt[:, off:LT],
                                lhsT=vt[:, t, :],
                                rhs=pt[:, LT * j + off : LT * (j + 1)],
                                start=(t == 0),
                                stop=(t == n_s - 1),
                            )
                        done += g

                    # epilogue: transpose back, normalize by row-sum
                    ovt = ovs.tile([D + 1, LT], BF16, tag="ov")
                    nc.vector.tensor_copy(ovt[:], pvt[:])
                    ost = ppv.tile([128, 4 * (D + 4)], BF16, tag="pv")
                    for j in range(4):
                        nc.tensor.transpose(
                            ost[:, (D + 4) * j : (D + 4) * j + D + 1],
                            ovt[:, 128 * j : 128 * (j + 1)],
                            identb[0 : D + 1, 0 : D + 1],
                        )
                    osr = ost[:].rearrange("p (j c) -> p j c", c=D + 4)
                    rt = rts.tile([128, 4], F32, tag="rt")
                    nc.vector.reciprocal(rt[:], osr[:, :, D])
                    nc.vector.tensor_mul(
                        outsb[:, 4 * lt : 4 * lt + 4, :],
                        osr[:, :, 0:D],
                        rt[:].unsqueeze(2).to_broadcast((128, 4, D)),
                    )
                nc.sync.dma_start(
                    out=o_d.ap()[h].rearrange("(c p) d -> p c d", p=128),
                    in_=outsb[:],
                )


def get_nc(reps=1):
    if reps not in _CACHE:
        _CACHE[reps] = _build_nc(reps)
    return _CACHE[reps]


def make_in_maps(q, k, v):
    q = np.ascontiguousarray(np.asarray(q, dtype=np.float32).reshape(B * H, S, D))
    k = np.ascontiguousarray(np.asarray(k, dtype=np.float32).reshape(B * H, S, D))
    v = np.ascontiguousarray(np.asarray(v, dtype=np.float32).reshape(B * H, S, D))
    maps = []
    for c in range(NCORES):
        sl = slice(c * NH, (c + 1) * NH)
        maps.append(
            {
                "q": np.ascontiguousarray(q[sl]),
                "k": np.ascontiguousarray(k[sl]),
                "v": np.ascontiguousarray(v[sl]),
            }
        )
    return maps


def kernel(q, k, v, attention_mask=None, **_ignored):
    """Full inputs in, full output out. attention_mask is all-ones by
    construction in this problem and drops out of the math."""
    from concourse.bass_utils import run_bass_kernel_spmd

    nc = get_nc()
    res = run_bass_kernel_spmd(nc, make_in_maps(q, k, v), core_ids=list(range(NCORES)))
    out = np.concatenate([res.results[c]["out"] for c in range(NCORES)], axis=0)
    return out.reshape(B, H, S, D).astype(np.float32)
